# revision 4
# baseline (speedup 1.0000x reference)
"""Trainium2 Bass kernel for nn_DenseGraphConvNodeToEdge — v4.

out[b,i,j,o] = y_rows[b,i,o] + base[b,j,o]
  base = y_cols + y_sum + bias;  y_rows = x @ W1.T

Producer split (PE column rate is fixed at 1.2G cols/s on this LNC —
fp8 DoubleRow measured at the same rate — so the PE alone floors at
~219us while the 64 MiB bf16 output DMA floors at ~190us):

  PE jblk  (25): K=65 bf16 matmul (x rows + ones) x (W1rep ; base row)
                 -> PSUM, copies to bf16 staging on ACT/DVE (greedy
                 load-balanced).
  OFF jblk (7) : flatten base -> [1,8192] row (gpsimd SWDGE), gpsimd
                 partition_broadcast -> [128,8192] (~12.6us, Pool is
                 otherwise idle), then 8 DVE tensor_adds [128,1024]
                 with y_rows free-dim stride-0 broadcast (2x mode,
                 ~0.66us each) into the staging tile. Split into 8 ops
                 so DVE never blocks the PSUM-copy stream for 4+us.

v3 -> v4 fixes (from per-instruction NTFF traces):
  * startup loads moved off the gpsimd SWDGE ring (w2tb/w1tb/xrt1b
    packets dribbled until t=16-25us, gating the prep chain); all
    constants now ride the sync/scalar HWDGE rings.
  * per-batch prep runs entirely at scheduler priority 0: the 8 base
    matmuls accumulate into ONE psum bank (start on j0, stop on j7,
    disjoint 64-col slices) and retire via ONE [128,512] copy, so
    prep never waits behind queued 1us staging copies. s2 shares that
    bank (its own start/stop group, copied out before j0's zeroing);
    y_rows borrows a psum_main tile (preps all run at startup).
  * bias folded into w2tb row 64 (v2's 256B bias DMA completed at
    t=18us); xsum row 64 memset to 1.0 picks it up in the s2 matmul.
  * ACT/DVE assignment of every copy/add by greedy accumulated-cost
    balance instead of a fixed pattern.

Output staged bf16 (rel ~2^-9; harness gate 2e-2) = 64 MiB/core.
"""

import numpy as np

B, N, C = 4, 1024, 64
N_CORES = 8
R = N // N_CORES  # 128 rows per core

# global jblk ids (b*8+jblk) produced by the broadcast+DVE pipeline.
# Kept out of {28..31} (per-group tail drain) and spread across b.
OFF_JBLKS = frozenset({4, 9, 14, 19, 22, 25, 27})

_CACHE = {}


def _build():
    import concourse.tile as tile
    from concourse import bacc, mybir

    f32 = mybir.dt.float32
    bf16 = mybir.dt.bfloat16

    nc = bacc.Bacc("TRN2", target_bir_lowering=False, debug=False,
                   num_devices=N_CORES)

    xt1b = nc.dram_tensor("xt1b", [C + 1, B * N], bf16, kind="ExternalInput").ap()
    xrt1b = nc.dram_tensor("xrt1b", [C + 1, B * R], bf16, kind="ExternalInput").ap()
    w1w = nc.dram_tensor("w1w", [C, 8192], bf16, kind="ExternalInput").ap()
    w0tb = nc.dram_tensor("w0tb", [C, C], bf16, kind="ExternalInput").ap()
    w1tb = nc.dram_tensor("w1tb", [C, C], bf16, kind="ExternalInput").ap()
    w2tb = nc.dram_tensor("w2tb", [C + 1, C], f32, kind="ExternalInput").ap()
    out_s = nc.dram_tensor("out_s", [B, R, N, C], bf16, kind="ExternalOutput").ap()

    with tile.TileContext(nc) as tc:
        with (
            tc.tile_pool(name="const", bufs=1) as const_pool,
            tc.tile_pool(name="rhs", bufs=1) as rhs_pool,
            tc.tile_pool(name="base", bufs=4) as base_pool,
            tc.tile_pool(name="yr", bufs=4) as yr_pool,
            tc.tile_pool(name="row", bufs=2) as row_pool,
            tc.tile_pool(name="bc", bufs=2) as bc_pool,
            tc.tile_pool(name="stage", bufs=3) as stage_pool,
            tc.tile_pool(name="psm", bufs=3, space="PSUM") as psum_main,
            tc.tile_pool(name="pss", bufs=1, space="PSUM") as psum_small,
        ):
            # ---- persistent SBUF state ----
            xt1_bf = const_pool.tile([C + 1, B * N], bf16, tag="xt1b")
            lhsT_sb = const_pool.tile([C + 1, B * R], bf16, tag="lhsT")
            rhs_base = [const_pool.tile([C + 1, C], bf16, tag=f"rhsb{k}",
                                        name=f"rhsb{k}")
                        for k in range(2)]
            w1t_sb = const_pool.tile([C, C], bf16, tag="w1t")
            w2t_sb = const_pool.tile([C + 1, C], f32, tag="w2t")
            xsum_sb = const_pool.tile([C + 1, 1], f32, tag="xsum")
            rhs_bufs = [rhs_pool.tile([C + 1, 8192], bf16, tag=f"rhs{k}",
                                      name=f"rhs{k}")
                        for k in range(3)]

            # xsum row 64 = 1.0 so the s2 matmul picks up the bias row of
            # w2tb; DVE is idle at startup
            nc.vector.memset(xsum_sb[C:C + 1, :], 1.0)

            # ---- input DMAs: HWDGE rings only, ordered by first use ----
            nc.sync.dma_start(xt1_bf[:, 0:N], xt1b[:, 0:N])
            nc.sync.dma_start(w2t_sb[:], w2tb[:, :])
            nc.sync.dma_start(w1t_sb[:], w1tb[:, :])
            nc.sync.dma_start(lhsT_sb[:], xrt1b[:, :])
            nc.sync.dma_start(xt1_bf[:, N:B * N], xt1b[:, N:B * N])
            nc.scalar.dma_start(rhs_base[0][0:C, :], w0tb[:, :])
            nc.scalar.dma_start(rhs_base[1][0:C, :], w0tb[:, :])
            nc.scalar.dma_start(rhs_bufs[0][:C, :], w1w[:, :])
            nc.scalar.dma_start(rhs_bufs[1][:C, :], w1w[:, :])
            nc.scalar.dma_start(rhs_bufs[2][:C, :], w1w[:, :])

            # greedy ACT/DVE load balance (est ns per op)
            eng_t = {"act": 0.0, "dve": 0.0}

            def assign(cost_act, cost_dve):
                if eng_t["act"] + cost_act <= eng_t["dve"] + cost_dve:
                    eng_t["act"] += cost_act
                    return "act"
                eng_t["dve"] += cost_dve
                return "dve"

            def copy_op(dst, src, cost_act, cost_dve):
                if assign(cost_act, cost_dve) == "act":
                    nc.scalar.copy(dst, src)
                else:
                    nc.vector.tensor_copy(dst, src)

            base_sb = {}
            yrows = {}

            def emit_prep(b):
                rb = rhs_base[b % 2]
                # xsum[c] = sum_j x[b,j,c] (bf16 in, f32 accumulate)
                nc.vector.reduce_sum(
                    xsum_sb[0:C, :], xt1_bf[0:C, b * N:(b + 1) * N],
                    axis=mybir.AxisListType.X)
                eng_t["dve"] += 1200.0
                # one psum bank for s2 + all 8 base tiles of this b
                big = psum_small.tile([128, 512], f32, tag="big",
                                      name=f"big_{b}")
                # s2_row[o] = xsum @ (W2.T ; bias) in exact fp32; its own
                # accumulation group, copied to rb row 64 before the base
                # matmuls re-zero the bank
                nc.tensor.matmul(big[0:1, 0:C], xsum_sb[:], w2t_sb[:],
                                 start=True, stop=True)
                copy_op(rb[C:C + 1, :], big[0:1, 0:C], 330.0, 250.0)
                # y_rows[b] = x_r @ W1.T  [128 i, 64 o]; borrows a psum_main
                # tile (preps all run at startup, before the main rotation)
                ps_y = psum_main.tile([128, 1024], f32, tag="psm")
                nc.tensor.matmul(
                    ps_y[:, 0:C], lhsT_sb[0:C, b * R:(b + 1) * R], w1t_sb[:],
                    start=True, stop=True)
                yt = yr_pool.tile([128, C], bf16, tag="yr", name=f"yr_{b}")
                copy_op(yt[:], ps_y[:, 0:C], 340.0, 260.0)
                yrows[b] = yt
                # base[b, :, :]: 8 matmuls accumulate into disjoint 64-col
                # slices of one zeroed bank (start on j0 zeroes the region)
                for jblk in range(8):
                    nc.tensor.matmul(
                        big[:, jblk * 64:(jblk + 1) * 64],
                        xt1_bf[:, b * N + jblk * 128: b * N + (jblk + 1) * 128],
                        rb[:],
                        start=(jblk == 0), stop=(jblk == 7),
                        skip_group_check=True)
                bt = base_pool.tile([128, 512], bf16, tag="base",
                                    name=f"base_{b}")
                copy_op(bt[:], big[:], 640.0, 680.0)
                base_sb[b] = bt

            with tc.high_priority():
                for b in range(B):
                    emit_prep(b)

            rhs_idx = 0    # rhs buffer rotation over PE-jblks only
            for b in range(B):
                lhsT = lhsT_sb[:, b * R:(b + 1) * R]
                for jblk in range(8):
                    g = b * 8 + jblk
                    j0 = jblk * 128
                    bt = base_sb[b][:, jblk * 64:(jblk + 1) * 64]
                    last = g >= 30

                    if g in OFF_JBLKS:
                        # broadcast+DVE producer
                        row_t = row_pool.tile([1, 8192], bf16, tag="row")
                        nc.gpsimd.dma_start(
                            row_t[:, :].rearrange("a (p o) -> a p o", p=128),
                            bt)
                        bc_t = bc_pool.tile([128, 8192], bf16, tag="bc")
                        nc.gpsimd.partition_broadcast(bc_t[:, :], row_t[:, :])
                        stage_t = stage_pool.tile([128, 8192], bf16,
                                                  tag="stage")
                        y_b = yrows[b][:, :].unsqueeze(1).broadcast_to(
                            (128, 16, C))
                        for grp in range(8):
                            sl = slice(grp * 1024, (grp + 1) * 1024)
                            nc.vector.tensor_add(
                                stage_t[:, sl].rearrange(
                                    "p (j o) -> p j o", j=16),
                                bc_t[:, sl].rearrange(
                                    "p (j o) -> p j o", j=16),
                                y_b)
                            eng_t["dve"] += 680.0
                        dma_eng = nc.sync if g % 2 == 0 else nc.scalar
                        dma_eng.dma_start(out_s[b, :, j0:j0 + 128, :],
                                          stage_t[:])
                        continue

                    rhs = rhs_bufs[rhs_idx % 3]
                    rhs_idx += 1
                    row64 = rhs[C:C + 1, :]
                    if g < 2:
                        # startup-latency-critical: halve the flatten into
                        # two concurrent 64-packet DMAs
                        nc.gpsimd.dma_start(
                            row64[:, 0:4096].rearrange("a (p o) -> a p o",
                                                       p=64),
                            base_sb[b][0:64, jblk * 64:(jblk + 1) * 64])
                        nc.sync.dma_start(
                            row64[:, 4096:8192].rearrange("a (p o) -> a p o",
                                                          p=64),
                            base_sb[b][64:128, jblk * 64:(jblk + 1) * 64])
                    else:
                        nc.gpsimd.dma_start(
                            row64[:, :].rearrange("a (p o) -> a p o", p=128),
                            bt)
                    stage_t = stage_pool.tile([128, 8192], bf16, tag="stage")
                    for grp in range(8):  # psum groups of [128, 1024]
                        ps_m = psum_main.tile([128, 1024], f32, tag="psm")
                        for h in range(2):
                            nc.tensor.matmul(
                                ps_m[:, h * 512:(h + 1) * 512],
                                lhsT,
                                rhs[:, grp * 1024 + h * 512:
                                    grp * 1024 + (h + 1) * 512],
                                start=True, stop=True)
                        dst = stage_t[:, grp * 1024:(grp + 1) * 1024]
                        copy_op(dst, ps_m[:], 1040.0, 1030.0)
                        if last:
                            # drain the final jblks as per-group DMAs split
                            # by partition halves across both rings
                            js = slice(j0 + grp * 16, j0 + (grp + 1) * 16)
                            nc.sync.dma_start(
                                out_s[b, 0:64, js, :], dst[0:64, :])
                            nc.scalar.dma_start(
                                out_s[b, 64:128, js, :], dst[64:128, :])
                    if not last:
                        dma_eng = nc.sync if g % 2 == 0 else nc.scalar
                        dma_eng.dma_start(out_s[b, :, j0:j0 + 128, :],
                                          stage_t[:])

    nc.compile()
    return nc


def _get_nc():
    if "nc" not in _CACHE:
        _CACHE["nc"] = _build()
    return _CACHE["nc"]


def _pack_inputs(x, W0, W1, W2, bias):
    import ml_dtypes

    bf = ml_dtypes.bfloat16
    x = np.ascontiguousarray(np.asarray(x, dtype=np.float32))
    W0 = np.asarray(W0, dtype=np.float32)
    W1 = np.asarray(W1, dtype=np.float32)
    W2 = np.asarray(W2, dtype=np.float32)
    bias = np.asarray(bias, dtype=np.float32)

    ones_n = np.ones((B, 1, N), dtype=np.float32)
    xt1b = np.ascontiguousarray(np.concatenate(
        [x.transpose(0, 2, 1), ones_n], axis=1).transpose(1, 0, 2)
        .reshape(C + 1, B * N).astype(bf))
    w1w = np.ascontiguousarray(np.tile(W1.T.astype(bf), (1, 128)))
    w0tb = np.ascontiguousarray(W0.T.astype(bf))
    w1tb = np.ascontiguousarray(W1.T.astype(bf))
    w2tb = np.ascontiguousarray(
        np.concatenate([W2.T, bias.T], axis=0))  # [65, 64] f32

    in_maps = []
    ones_r = np.ones((B, 1, R), dtype=np.float32)
    for c in range(N_CORES):
        xr = x[:, c * R:(c + 1) * R, :]
        xrt1b = np.ascontiguousarray(np.concatenate(
            [xr.transpose(0, 2, 1), ones_r], axis=1).transpose(1, 0, 2)
            .reshape(C + 1, B * R).astype(bf))
        in_maps.append({
            "xt1b": xt1b, "xrt1b": xrt1b, "w1w": w1w,
            "w0tb": w0tb, "w1tb": w1tb, "w2tb": w2tb,
        })
    return in_maps


def kernel(x, adj, W0, W1, W2, bias):
    from concourse.bass_utils import run_bass_kernel_spmd

    nc = _get_nc()
    in_maps = _pack_inputs(x, W0, W1, W2, bias)

    global _last_in_maps
    _last_in_maps = in_maps
    res = run_bass_kernel_spmd(nc, in_maps, list(range(N_CORES)))

    out = np.empty((B, N, N, C), dtype=np.float32)
    for c in range(N_CORES):
        out[:, c * R:(c + 1) * R] = np.asarray(
            res.results[c]["out_s"]).astype(np.float32)
    return out
